# revision 40
# baseline (speedup 1.0000x reference)
"""Trainium2 Bass kernel for nn_DecoderBlock (dynamic-conv decoder block).

Data-parallel over batch: 16 samples -> 8 cores x 2 samples.
All weights replicated, host-repacked to bf16 in matmul/DMA-friendly layouts.

Math per sample (C=512, G=64, cg=8, H=W=32, S=512, Cout=256):
  dw   = conv3x3(reflect_pad(w), kp_sw) + kp_sb        # kernel predictor, per-sample
  pw   = pooled @ kp_pw.T + kp_pb ;  bias = pooled @ kp_bw.T + kp_bb
  xn   = instance_norm(x)
  y    = grouped_dynconv3x3(reflect_pad(xn), dw)       # per-sample weights
  y    = grouped_pointwise(pw, y) + bias
  y    = relu(conv3x3(y, dec_w1) + b1)
  y    = relu(conv3x3(y, dec_w2) + b2)
  out  = nearest_upsample_2x(y)

All matmuls run bf16 x bf16 with f32 PSUM accumulation (rel err ~6e-3).

The kernel-predictor spatial branch (the 38 MB kp_sw matmul) is sharded over
the 8 cores by output column: each core computes its 512-col slice for ALL 16
samples, then one 147 KB AllToAll redistributes so each core holds the full
4096-col predictor output for its own 2 samples. The pointwise/bias heads stay
local. The style-map im2col and pooled means are precomputed on host.

Grouped convs use block-diagonal [128,128] weight tiles built on device by
scattering predictor output through per-(sample,tile) DRAM scratch slots with
a diagonal-embedding view: slot readback is [128 rows x 1290] (slot allocated
128x1291 so SLOT/16 = 8*1290+8 is the group stride = 8 partitions down + 8
cols right); tap p of 10 (9 spatial + 1 pointwise) sits at column 129*p.
DMA access patterns cap at 3 dims, so the scatter goes one tap at a time.
"""

import sys

sys.path.insert(0, "/opt/trn_rl_repo")

import numpy as np
import ml_dtypes

import concourse.bacc as bacc
import concourse.bass as bass
import concourse.tile as tile
from concourse import mybir
from concourse.alu_op_type import AluOpType
from concourse.bass_utils import run_bass_kernel_spmd

F32 = mybir.dt.float32
BF16 = mybir.dt.bfloat16
AF = mybir.ActivationFunctionType

NCORES = 8
BPC = 2          # samples per core
C = 512          # in channels
CO = 256         # out channels
S = 512          # style dim
G = 64           # groups
CG = 8           # channels per group
H = W = 32
HW = H * W
NT = C // 128    # 4 channel tiles
NM2 = CO // 128  # 2 out-channel tiles
EPS = 1e-5
BALL = NCORES * BPC           # full batch (predictor is computed for all of it)
NSH = (C * CG) // NCORES      # 512: this core's slice of predictor output cols
ROW = 10 * 129               # 1290: per-partition readback row of a scratch slot
SLOT = 128 * (ROW + 1)        # slot is 128*(ROW+1) so that SLOT/16 = 8*ROW + 8,
# the diagonal-embedding group stride (8 partitions down + 8 cols right). The
# readback views only the first 128*ROW elements with row stride ROW.

_CACHE = {}
DEBUG = False


def _slot_diag_pos(scr, pos):
    """Diagonal-embedding view of a scratch slot for tap `pos` (0..9):
    dims (g 16, i 8, o 8), strides (8*ROW+8, ROW, 1) + offset 129*pos.
    Slice-then-split keeps the +8 column offset in the group stride."""
    v = scr.rearrange("(g x) -> g x", g=16)            # g stride 8*ROW+8
    v = v[:, : CG * ROW].rearrange("g (i r) -> g i r", i=CG)   # i stride ROW
    v = v.rearrange("g i (pos c) -> g i pos c", pos=10)        # pos stride 129
    return v[:, :, pos, :CG]





def _build():
    nc = bacc.Bacc(None, target_bir_lowering=False)

    x2 = nc.declare_dram_parameter("x2", [BPC, C, H, W], F32, isOutput=False)
    xwh = nc.declare_dram_parameter("xwh", [S, 9 * BALL * 9], BF16, isOutput=False)
    plh = nc.declare_dram_parameter("plh", [S, BALL], BF16, isOutput=False)
    plo = nc.declare_dram_parameter("plo", [S, BPC], BF16, isOutput=False)
    kpsw = nc.declare_dram_parameter("kpsw", [9 * S + 1, NSH], BF16, isOutput=False)
    kppw = nc.declare_dram_parameter("kppw", [S + 1, C * CG], BF16, isOutput=False)
    kpbw = nc.declare_dram_parameter("kpbw", [S + 1, C], BF16, isOutput=False)
    w1p = nc.declare_dram_parameter("w1p", [NT, NT, 128, 9 * 128], BF16, isOutput=False)
    w2p = nc.declare_dram_parameter("w2p", [NT, NM2, 128, 9 * 128], BF16, isOutput=False)
    b1d = nc.declare_dram_parameter("b1d", [C], F32, isOutput=False)
    b2d = nc.declare_dram_parameter("b2d", [CO], F32, isOutput=False)
    yout = nc.declare_dram_parameter("yout", [BPC, CO, 2 * H, 2 * W], F32, isOutput=True)
    if DEBUG:
        dbg_dwTP = nc.declare_dram_parameter("dbg_dwTP", [18, C * CG], BF16, isOutput=True)
        dbg_pwT = nc.declare_dram_parameter("dbg_pwT", [BPC, C * CG], BF16, isOutput=True)
        dbg_dwb = nc.declare_dram_parameter("dbg_dwb", [128, ROW], BF16, isOutput=True)
        dbg_yp1 = nc.declare_dram_parameter("dbg_yp1", [128, NT, 34, 34], BF16, isOutput=True)
        dbg_biasc = nc.declare_dram_parameter("dbg_biasc", [128, NT, BPC], F32, isOutput=True)

    # one scratch slot per (sample, channel-tile) to keep DMA deps precise
    scr = [[nc.dram_tensor(f"dwscr{b}{t}", [SLOT], BF16) for t in range(NT)]
           for b in range(BPC)]
    # all-to-all staging: 8 blocks x 18 dw rows x 512 cols
    a2ain = nc.dram_tensor("a2ain", [NCORES * 18, NSH], BF16)
    a2aout = nc.dram_tensor("a2aout", [NCORES * 18, NSH], BF16)

    with tile.TileContext(nc) as tc:
        with (
            tc.tile_pool(name="consts", bufs=1) as consts,
            tc.tile_pool(name="stream", bufs=6) as stream,
            tc.tile_pool(name="xin", bufs=2) as xin,
            tc.tile_pool(name="actp", bufs=3) as actp,
            tc.tile_pool(name="wstream", bufs=6) as wstream,
            tc.tile_pool(name="blk", bufs=8) as blkp,
            tc.tile_pool(name="pad3", bufs=5) as pad3,
            tc.tile_pool(name="dwtp", bufs=1) as dwtp,
            tc.tile_pool(name="outp", bufs=2) as outp,
            tc.tile_pool(name="psum", bufs=6, space="PSUM") as psum,
            tc.tile_pool(name="psumb", bufs=2, space="PSUM") as psumb,
        ):
            # ---------------- persistent small constants ----------------
            epsb = consts.tile([128, 1], F32, tag="epsb")
            nc.vector.memset(epsb[:], EPS)
            zbias = consts.tile([128, 1], F32, tag="zbias")
            nc.vector.memset(zbias[:], 0.0)
            onesf = consts.tile([1, 80], F32, tag="onesf")
            nc.vector.memset(onesf[:], 1.0)
            ones = consts.tile([1, 80], BF16, tag="ones")
            nc.vector.tensor_copy(ones[:], onesf[:])
            zbf = consts.tile([128, ROW + 1], BF16, tag="zbf")
            nc.vector.memset(zbf[:], 0.0)

            # zero-fill all scratch slots (issued early, on DVE's DGE)
            for b in range(BPC):
                for t in range(NT):
                    nc.gpsimd.dma_start(
                        out=scr[b][t].rearrange("(p r) -> p r", p=128),
                        in_=zbf[:, :],
                    )

            # ---------------- style im2col / pooled (host-prepared) ----------------
            # xw[p, q, pos, (b,i,j)]: lhsT slices for the dw predictor matmuls
            xw = consts.tile([128, 4, 9, BALL * 9], BF16, tag="xw")
            nc.scalar.dma_start(
                out=xw[:, :, :, :].rearrange("p q pos f -> p q (pos f)"),
                in_=xwh.rearrange("(q p) f -> p q f", p=128),
            )
            pooled = consts.tile([128, 4, BALL], BF16, tag="pooled")
            nc.scalar.dma_start(
                out=pooled[:, :, :], in_=plh.rearrange("(q p) b -> p q b", p=128)
            )
            pooledo = consts.tile([128, 4, BPC], BF16, tag="pooledo")
            nc.scalar.dma_start(
                out=pooledo[:, :, :], in_=plo.rearrange("(q p) b -> p q b", p=128)
            )

            # ------- predictor dw: this core's 512-col slice for ALL 16 samples -------
            # M layout: (sample 16, pos 9) = 144 rows, split into two 72-row psums.
            kswb = consts.tile([1, NSH], BF16, tag="kswb")
            nc.sync.dma_start(out=kswb[:, :], in_=kpsw[9 * S : 9 * S + 1, :])
            psA = psum.tile([72, 512], F32, tag="mm", name="psA")
            psB = psum.tile([72, 512], F32, tag="mm", name="psB")
            for pos in range(9):
                kst = stream.tile([128, 4, 512], BF16, tag="kst")
                nc.sync.dma_start(
                    out=kst[:, :, :],
                    in_=kpsw[pos * S : (pos + 1) * S, :].rearrange(
                        "(q p) c -> p q c", p=128
                    ),
                )
                for q in range(4):
                    first = pos == 0 and q == 0
                    nc.tensor.matmul(
                        psA[:], xw[:, q, pos, :72], kst[:, q, :],
                        start=first, stop=False,
                    )
                    nc.tensor.matmul(
                        psB[:], xw[:, q, pos, 72:144], kst[:, q, :],
                        start=first, stop=False,
                    )
            nc.tensor.matmul(psA[:], ones[:1, :72], kswb[:1, :], start=False, stop=True)
            nc.tensor.matmul(psB[:], ones[:1, :72], kswb[:1, :], start=False, stop=True)

            # stage into all-to-all block layout (18 dw rows per dest block)
            st0 = dwtp.tile([72, 512], BF16, tag="st0")
            st1 = dwtp.tile([72, 512], BF16, tag="st1")
            nc.vector.tensor_copy(st0[:, :], psA[:])
            nc.vector.tensor_copy(st1[:, :], psB[:])
            nc.scalar.dma_start(out=a2ain[:72, :], in_=st0[:, :])
            nc.scalar.dma_start(out=a2ain[72:, :], in_=st1[:, :])

            # exchange: after this, block s of a2aout holds cols [512s, 512s+512)
            # of dw for THIS core's two samples
            nc.gpsimd.collective_compute(
                kind="AllToAll",
                op=AluOpType.bypass,
                replica_groups=[list(range(NCORES))],
                ins=[a2ain[:, :]],
                outs=[a2aout[:, :]],
            )

            # ------- pw predictor: local (own 2 samples, full 4096 cols) -------
            # runs on the PE while the collective is in flight
            kppwb = consts.tile([1, C * CG], BF16, tag="kppwb")
            nc.sync.dma_start(out=kppwb[:, :], in_=kppw[S : S + 1, :])
            pwT = dwtp.tile([BPC, C * CG], BF16, tag="pwT")
            for n in range(8):
                ncol = slice(512 * n, 512 * (n + 1))
                kpt = stream.tile([128, 4, 512], BF16, tag="kst", name=f"kpt{n}")
                nc.sync.dma_start(
                    out=kpt[:, :, :],
                    in_=kppw[:S, ncol].rearrange("(q p) c -> p q c", p=128),
                )
                ps2 = psum.tile([BPC, 512], F32, tag="mm", name=f"pw{n}")
                for q in range(4):
                    nc.tensor.matmul(
                        ps2[:], pooledo[:, q, :], kpt[:, q, :], start=(q == 0), stop=False
                    )
                nc.tensor.matmul(
                    ps2[:], ones[:1, :BPC], kppwb[:1, ncol], start=False, stop=True
                )
                nc.vector.tensor_copy(pwT[:, ncol], ps2[:])

            # bias head (local): biasc[c, t, b] from pooled-own @ kp_bw.T + kp_bb
            kpbwb = consts.tile([1, C], BF16, tag="kpbwb")
            nc.sync.dma_start(out=kpbwb[:, :], in_=kpbw[S : S + 1, :])
            biasc = consts.tile([128, NT, BPC], F32, tag="biasc")
            for m in range(NT):
                kbt = stream.tile([128, 4, 128], BF16, tag="kbt")
                nc.sync.dma_start(
                    out=kbt[:, :, :],
                    in_=kpbw[:S, 128 * m : 128 * (m + 1)].rearrange(
                        "(q p) c -> p q c", p=128
                    ),
                )
                ps = psumb.tile([128, BPC], F32, tag="mmb")
                for q in range(4):
                    nc.tensor.matmul(
                        ps[:], kbt[:, q, :], pooledo[:, q, :], start=(q == 0), stop=False
                    )
                nc.tensor.matmul(
                    ps[:],
                    kpbwb[:1, 128 * m : 128 * (m + 1)],
                    ones[:1, :BPC],
                    start=False,
                    stop=True,
                )
                nc.vector.tensor_copy(biasc[:, m, :], ps[:])

            # ---------------- x load + instance norm -> padded bf16 ----------------
            xps = []
            for b in range(BPC):
                xbig = xin.tile([128, NT, HW], F32, tag="xsb")
                nc.sync.dma_start(
                    out=xbig[:, :, :],
                    in_=x2[b].rearrange("(t p) h w -> p t (h w)", p=128),
                )
                xp = pad3.tile([128, NT, 34, 34], BF16, tag="padbuf")
                xps.append(xp)
                for t in range(NT):
                    st = actp.tile([128, 2, 6], F32, tag="bnst")
                    xsb2 = xbig[:, t, :].rearrange("p (s f) -> p s f", f=512)
                    for sg in range(2):
                        nc.vector.bn_stats(out=st[:, sg, :], in_=xsb2[:, sg, :])
                    mv = actp.tile([128, 2], F32, tag="bnmv")
                    nc.vector.bn_aggr(out=mv[:], in_=st[:])
                    rstd = actp.tile([128, 1], F32, tag="rstd")
                    nc.scalar.activation(
                        out=rstd[:], in_=mv[:, 1:2], func=AF.Sqrt, bias=epsb[:], scale=1.0
                    )
                    nc.vector.reciprocal(out=rstd[:], in_=rstd[:])
                    nc.vector.tensor_scalar(
                        out=xp[:, t, 1:33, 1:33],
                        in0=xbig[:, t, :].rearrange("p (h w) -> p h w", h=H),
                        scalar1=mv[:, 0:1],
                        scalar2=rstd[:],
                        op0=AluOpType.subtract,
                        op1=AluOpType.mult,
                    )
                    nc.vector.tensor_copy(xp[:, t, 1:33, 0:1], xp[:, t, 1:33, 2:3])
                    nc.vector.tensor_copy(xp[:, t, 1:33, 33:34], xp[:, t, 1:33, 31:32])
                    nc.vector.tensor_copy(xp[:, t, 0, :], xp[:, t, 2, :])
                    nc.vector.tensor_copy(xp[:, t, 33, :], xp[:, t, 31, :])

            # assemble dw rows per tile, scatter into the diagonal slots
            dwTP = dwtp.tile([2 * 9, C * CG], BF16, tag="dwtp")
            dwbs = {}
            for t in range(NT):
                tcol = slice(1024 * t, 1024 * (t + 1))
                nc.sync.dma_start(
                    out=dwTP[:, tcol].rearrange("r (s c) -> r s c", s=2),
                    in_=a2aout.rearrange("(s r) c -> r s c", s=NCORES)[
                        :, 2 * t : 2 * t + 2, :
                    ],
                )
                for b in range(BPC):
                    for pos in range(9):
                        eng = nc.sync if pos % 2 == 0 else nc.scalar
                        eng.dma_start(
                            out=_slot_diag_pos(scr[b][t], pos),
                            in_=dwTP[9 * b + pos : 9 * b + pos + 1, tcol].rearrange(
                                "p (g i o) -> p g i o", g=16, i=CG
                            ),
                        )
                    nc.scalar.dma_start(
                        out=_slot_diag_pos(scr[b][t], 9),
                        in_=pwT[b : b + 1, tcol].rearrange(
                            "p (g i o) -> p g i o", g=16, i=CG
                        ),
                    )
                    dwb = blkp.tile([128, ROW], BF16, tag="dwb", name=f"dwb{b}{t}")
                    nc.sync.dma_start(
                        out=dwb[:, :],
                        in_=scr[b][t][: 128 * ROW].rearrange("(p r) -> p r", p=128),
                    )
                    dwbs[(b, t)] = dwb

            if DEBUG:
                nc.sync.dma_start(out=dbg_dwTP[:, :], in_=dwTP[:, :])
                nc.sync.dma_start(out=dbg_pwT[:, :], in_=pwT[:, :])
                nc.sync.dma_start(out=dbg_dwb[:, :], in_=dwbs[(0, 0)][:, :])
                nc.sync.dma_start(out=dbg_biasc[:, :, :], in_=biasc[:, :, :])

            # ---------------- phase A: adaconv (dynamic grouped conv + pointwise) ----
            yp1s = []
            for b in range(BPC):
                yp1 = pad3.tile([128, NT, 34, 34], BF16, tag="padbuf")
                yp1s.append(yp1)
                for t in range(NT):
                    nc.vector.tensor_copy(yp1[:, t, 0, :], zbf[:, :34])
                    nc.vector.tensor_copy(yp1[:, t, 33, :], zbf[:, :34])
                    nc.vector.tensor_copy(
                        yp1[:, t, 1:33, 0:1],
                        zbf[:, :32].rearrange("p (a c) -> p a c", c=1),
                    )
                    nc.vector.tensor_copy(
                        yp1[:, t, 1:33, 33:34],
                        zbf[:, :32].rearrange("p (a c) -> p a c", c=1),
                    )
            for t in range(NT):
                for b in range(BPC):
                    dwb = dwbs[(b, t)]
                    ysb = actp.tile([128, HW], BF16, tag="ysb")
                    for hh in range(2):
                        ps = psum.tile([128, 512], F32, tag="mm")
                        for kdi in range(3):
                            for kdj in range(3):
                                pos = kdi * 3 + kdj
                                nc.tensor.matmul(
                                    ps[:],
                                    dwb[:, 129 * pos : 129 * pos + 128],
                                    xps[b][:, t, kdi + 16 * hh : kdi + 16 * hh + 16, kdj : kdj + 32],
                                    start=(pos == 0),
                                    stop=(pos == 8),
                                )
                        nc.vector.tensor_copy(ysb[:, 512 * hh : 512 * (hh + 1)], ps[:])
                    for hh in range(2):
                        ps2 = psum.tile([128, 512], F32, tag="mm")
                        nc.tensor.matmul(
                            ps2[:],
                            dwb[:, 129 * 9 : 129 * 9 + 128],
                            ysb[:, 512 * hh : 512 * (hh + 1)],
                            start=True,
                            stop=True,
                        )
                        nc.scalar.activation(
                            out=yp1s[b][:, t, 1 + 16 * hh : 17 + 16 * hh, 1:33],
                            in_=ps2[:].rearrange("p (h w) -> p h w", h=16),
                            func=AF.Identity,
                            bias=biasc[:, t, b : b + 1],
                            scale=1.0,
                        )

            if DEBUG:
                nc.sync.dma_start(out=dbg_yp1[:, :, :, :], in_=yp1s[0][:, :, :, :])

            b1sb = consts.tile([128, NT], F32, tag="b1sb")
            nc.sync.dma_start(out=b1sb[:, :], in_=b1d.rearrange("(m c) -> c m", c=128))
            b2sb = consts.tile([128, NM2], F32, tag="b2sb")
            nc.sync.dma_start(out=b2sb[:, :], in_=b2d.rearrange("(m c) -> c m", c=128))

            # ---------------- phase B: conv1 (512 -> 512) + relu ----------------
            yp2s = []
            for b in range(BPC):
                yp2 = pad3.tile([128, NT, 34, 34], BF16, tag="padbuf")
                yp2s.append(yp2)
                for m in range(NT):
                    nc.vector.tensor_copy(yp2[:, m, 0, :], zbf[:, :34])
                    nc.vector.tensor_copy(yp2[:, m, 33, :], zbf[:, :34])
                    nc.vector.tensor_copy(
                        yp2[:, m, 1:33, 0:1],
                        zbf[:, :32].rearrange("p (a c) -> p a c", c=1),
                    )
                    nc.vector.tensor_copy(
                        yp2[:, m, 1:33, 33:34],
                        zbf[:, :32].rearrange("p (a c) -> p a c", c=1),
                    )
            for b in range(BPC):
                for m in range(NT):
                    pss = [psum.tile([128, 512], F32, tag="mm", name=f"pss{b}_{m}_{i}") for i in range(2)]
                    for k in range(NT):
                        w1k = wstream.tile([128, 9 * 128], BF16, tag="ws")
                        nc.sync.dma_start(out=w1k[:, :], in_=w1p[k, m])
                        for hh in range(2):
                            ps = pss[hh]
                            for kdi in range(3):
                                for kdj in range(3):
                                    pos = kdi * 3 + kdj
                                    nc.tensor.matmul(
                                        ps[:],
                                        w1k[:, 128 * pos : 128 * (pos + 1)],
                                        yp1s[b][:, k, kdi + 16 * hh : kdi + 16 * hh + 16, kdj : kdj + 32],
                                        start=(k == 0 and pos == 0),
                                        stop=(k == NT - 1 and pos == 8),
                                    )
                    for hh in range(2):
                        nc.scalar.activation(
                            out=yp2s[b][:, m, 1 + 16 * hh : 17 + 16 * hh, 1:33],
                            in_=pss[hh][:].rearrange("p (h w) -> p h w", h=16),
                            func=AF.Relu,
                            bias=b1sb[:, m : m + 1],
                            scale=1.0,
                        )

            # ------- phase C: conv2 (512 -> 256) + relu + 2x nearest upsample -------
            for b in range(BPC):
                for m2 in range(NM2):
                    pss = [psum.tile([128, 512], F32, tag="mm", name=f"psc{b}_{m2}_{i}") for i in range(2)]
                    for k in range(NT):
                        w2k = wstream.tile([128, 9 * 128], BF16, tag="ws")
                        nc.sync.dma_start(out=w2k[:, :], in_=w2p[k, m2])
                        for hh in range(2):
                            ps = pss[hh]
                            for kdi in range(3):
                                for kdj in range(3):
                                    pos = kdi * 3 + kdj
                                    nc.tensor.matmul(
                                        ps[:],
                                        w2k[:, 128 * pos : 128 * (pos + 1)],
                                        yp2s[b][:, k, kdi + 16 * hh : kdi + 16 * hh + 16, kdj : kdj + 32],
                                        start=(k == 0 and pos == 0),
                                        stop=(k == NT - 1 and pos == 8),
                                    )
                    for hh in range(2):
                        ps = pss[hh]
                        # ous free dim = (h 16, two_h 2, w 32, two_w 2) = 2048
                        ous = outp.tile([128, 16, 2, 32, 2], F32, tag="ous")
                        for a in range(2):
                            for a2 in range(2):
                                nc.scalar.activation(
                                    out=ous[:, :, a, :, a2],
                                    in_=ps[:].rearrange("p (h w) -> p h w", h=16),
                                    func=AF.Relu,
                                    bias=b2sb[:, m2 : m2 + 1],
                                    scale=1.0,
                                )
                        nc.sync.dma_start(
                            out=yout[b, 128 * m2 : 128 * (m2 + 1)]
                            .rearrange("c h w -> c (h w)")[
                                :, 2048 * hh : 2048 * (hh + 1)
                            ]
                            .rearrange("c (h r) -> c h r", h=16),
                            in_=ous[:, :, :, :, :].rearrange("p h th w tw -> p h (th w tw)"),
                        )

    nc.compile()
    return nc


def _repack(inputs):
    bf = ml_dtypes.bfloat16
    kp_sw = np.ascontiguousarray(inputs["kp_sw"], dtype=np.float32)
    kp_sb = np.ascontiguousarray(inputs["kp_sb"], dtype=np.float32)
    kp_pw = np.ascontiguousarray(inputs["kp_pw"], dtype=np.float32)
    kp_pb = np.ascontiguousarray(inputs["kp_pb"], dtype=np.float32)
    kp_bw = np.ascontiguousarray(inputs["kp_bw"], dtype=np.float32)
    kp_bb = np.ascontiguousarray(inputs["kp_bb"], dtype=np.float32)
    dec_w1 = np.ascontiguousarray(inputs["dec_w1"], dtype=np.float32)
    dec_b1 = np.ascontiguousarray(inputs["dec_b1"], dtype=np.float32)
    dec_w2 = np.ascontiguousarray(inputs["dec_w2"], dtype=np.float32)
    dec_b2 = np.ascontiguousarray(inputs["dec_b2"], dtype=np.float32)

    # column permutation: new col (t, g, i, o) <- original row (g*8+o)*8 + i
    O = np.arange(C * CG).reshape(NT, 16, CG, CG)  # (t, g, o, i), flat-major
    P = O.transpose(0, 1, 3, 2).reshape(-1)        # (t, g, i, o)

    kpsw = np.empty((9 * S + 1, C * CG), dtype=bf)
    kpsw[: 9 * S] = (
        kp_sw[P].reshape(C * CG, S, 3, 3).transpose(2, 3, 1, 0).reshape(9 * S, C * CG)
    ).astype(bf)  # rows in k-order (di, dj, s)
    kpsw[9 * S] = kp_sb[P].astype(bf)
    kpsw_sl = [np.ascontiguousarray(kpsw[:, NSH * c : NSH * (c + 1)])
               for c in range(NCORES)]

    kppw = np.empty((S + 1, C * CG), dtype=bf)
    kppw[:S] = kp_pw[P].T.astype(bf)
    kppw[S] = kp_pb[P].astype(bf)

    kpbw = np.empty((S + 1, C), dtype=bf)
    kpbw[:S] = kp_bw.T.astype(bf)
    kpbw[S] = kp_bb.astype(bf)

    # decoder weights: [k, m, p, (pos, c)] fully contiguous per (k, m) tile
    w1t = dec_w1.transpose(2, 3, 1, 0).reshape(9, NT, 128, NT, 128)
    w1p = np.ascontiguousarray(w1t.transpose(1, 3, 2, 0, 4).reshape(NT, NT, 128, 9 * 128)).astype(bf)
    w2t = dec_w2.transpose(2, 3, 1, 0).reshape(9, NT, 128, NM2, 128)
    w2p = np.ascontiguousarray(w2t.transpose(1, 3, 2, 0, 4).reshape(NT, NM2, 128, 9 * 128)).astype(bf)

    shared = {
        "kppw": kppw,
        "kpbw": kpbw,
        "w1p": w1p,
        "w2p": w2p,
        "b1d": dec_b1,
        "b2d": dec_b2,
    }
    return shared, kpsw_sl


def _style_prep(w):
    """Host-side style-map im2col (reflect-padded) + pooled means, bf16."""
    bf = ml_dtypes.bfloat16
    wf = np.ascontiguousarray(w, dtype=np.float32)
    wpad = np.pad(wf, ((0, 0), (0, 0), (1, 1), (1, 1)), mode="reflect")
    xwh = np.empty((S, 9, BALL * 9), np.float32)
    for di in range(3):
        for dj in range(3):
            win = wpad[:, :, di : di + 3, dj : dj + 3]
            xwh[:, di * 3 + dj, :] = win.transpose(1, 0, 2, 3).reshape(S, BALL * 9)
    pl = np.ascontiguousarray(wf.mean(axis=(2, 3)).T)
    return (
        np.ascontiguousarray(xwh.reshape(S, 9 * BALL * 9)).astype(bf),
        pl.astype(bf),
    )


def kernel(**inputs):
    if "nc" not in _CACHE:
        _CACHE["nc"] = _build()
    nc = _CACHE["nc"]

    shared, kpsw_sl = _repack(inputs)
    x = np.ascontiguousarray(inputs["x"], dtype=np.float32)
    w = np.ascontiguousarray(inputs["w"], dtype=np.float32)
    xwh, pl = _style_prep(w)

    in_maps = []
    for c in range(NCORES):
        sl = slice(BPC * c, BPC * (c + 1))
        in_maps.append({
            "x2": x[sl], "xwh": xwh, "plh": pl,
            "plo": np.ascontiguousarray(pl[:, sl]),
            "kpsw": kpsw_sl[c], **shared,
        })

    res = run_bass_kernel_spmd(nc, in_maps, list(range(NCORES))).results
    return np.concatenate([r["yout"] for r in res], axis=0)


# revision 43
# speedup vs baseline: 1.2786x; 1.2786x over previous
"""Trainium2 Bass kernel for nn_DecoderBlock (dynamic-conv decoder block).

Data-parallel over batch: 16 samples -> 8 cores x 2 samples.
All weights replicated, host-repacked to bf16 in matmul/DMA-friendly layouts.

Math per sample (C=512, G=64, cg=8, H=W=32, S=512, Cout=256):
  dw   = conv3x3(reflect_pad(w), kp_sw) + kp_sb        # kernel predictor, per-sample
  pw   = pooled @ kp_pw.T + kp_pb ;  bias = pooled @ kp_bw.T + kp_bb
  xn   = instance_norm(x)
  y    = grouped_dynconv3x3(reflect_pad(xn), dw)       # per-sample weights
  y    = grouped_pointwise(pw, y) + bias
  y    = relu(conv3x3(y, dec_w1) + b1)
  y    = relu(conv3x3(y, dec_w2) + b2)
  out  = nearest_upsample_2x(y)

All matmuls run bf16 x bf16 with f32 PSUM accumulation (rel err ~6e-3).

The kernel-predictor spatial branch (the 38 MB kp_sw matmul) is sharded over
the 8 cores by output column: each core computes its 512-col slice for ALL 16
samples, then one 147 KB AllToAll redistributes so each core holds the full
4096-col predictor output for its own 2 samples. The pointwise/bias heads stay
local. The style-map im2col and pooled means are precomputed on host.

Grouped convs use block-diagonal [128,128] weight tiles built on device by
scattering predictor output through per-(sample,tile) DRAM scratch slots with
a diagonal-embedding view: slot readback is [128 rows x 1290] (slot allocated
128x1291 so SLOT/16 = 8*1290+8 is the group stride = 8 partitions down + 8
cols right); tap p of 10 (9 spatial + 1 pointwise) sits at column 129*p.
DMA access patterns cap at 3 dims, so the scatter goes one tap at a time.
"""

import sys

sys.path.insert(0, "/opt/trn_rl_repo")

import numpy as np
import ml_dtypes

import concourse.bacc as bacc
import concourse.bass as bass
import concourse.tile as tile
from concourse import mybir
from concourse.alu_op_type import AluOpType
from concourse.bass_utils import run_bass_kernel_spmd

F32 = mybir.dt.float32
BF16 = mybir.dt.bfloat16
AF = mybir.ActivationFunctionType

NCORES = 8
BPC = 2          # samples per core
C = 512          # in channels
CO = 256         # out channels
S = 512          # style dim
G = 64           # groups
CG = 8           # channels per group
H = W = 32
HW = H * W
NT = C // 128    # 4 channel tiles
NM2 = CO // 128  # 2 out-channel tiles
EPS = 1e-5
BALL = NCORES * BPC           # full batch (predictor is computed for all of it)
NSH = (C * CG) // NCORES      # 512: this core's slice of predictor output cols
ROW = 10 * 129               # 1290: per-partition readback row of a scratch slot
SLOT = 128 * (ROW + 1)        # slot is 128*(ROW+1) so that SLOT/16 = 8*ROW + 8,
# the diagonal-embedding group stride (8 partitions down + 8 cols right). The
# readback views only the first 128*ROW elements with row stride ROW.

_CACHE = {}
DEBUG = False


def _slot_diag_pos(scr, pos):
    """Diagonal-embedding view of a scratch slot for tap `pos` (0..9):
    dims (g 16, i 8, o 8), strides (8*ROW+8, ROW, 1) + offset 129*pos.
    Slice-then-split keeps the +8 column offset in the group stride."""
    v = scr.rearrange("(g x) -> g x", g=16)            # g stride 8*ROW+8
    v = v[:, : CG * ROW].rearrange("g (i r) -> g i r", i=CG)   # i stride ROW
    v = v.rearrange("g i (pos c) -> g i pos c", pos=10)        # pos stride 129
    return v[:, :, pos, :CG]





def _build():
    nc = bacc.Bacc(None, target_bir_lowering=False)

    x2 = nc.declare_dram_parameter("x2", [BPC, C, H, W], F32, isOutput=False)
    xwh = nc.declare_dram_parameter("xwh", [S, 9 * BALL * 9], BF16, isOutput=False)
    plh = nc.declare_dram_parameter("plh", [S, BALL], BF16, isOutput=False)
    plo = nc.declare_dram_parameter("plo", [S, BPC], BF16, isOutput=False)
    kpsw = nc.declare_dram_parameter("kpsw", [9 * S + 1, NSH], BF16, isOutput=False)
    kppw = nc.declare_dram_parameter("kppw", [S + 1, C * CG], BF16, isOutput=False)
    kpbw = nc.declare_dram_parameter("kpbw", [S + 1, C], BF16, isOutput=False)
    w1p = nc.declare_dram_parameter("w1p", [NT, NT, 128, 9 * 128], BF16, isOutput=False)
    w2p = nc.declare_dram_parameter("w2p", [NT, NM2, 128, 9 * 128], BF16, isOutput=False)
    b1d = nc.declare_dram_parameter("b1d", [C], F32, isOutput=False)
    b2d = nc.declare_dram_parameter("b2d", [CO], F32, isOutput=False)
    yout = nc.declare_dram_parameter("yout", [BPC, CO, 2 * H, 2 * W], F32, isOutput=True)
    if DEBUG:
        dbg_dwTP = nc.declare_dram_parameter("dbg_dwTP", [18, C * CG], BF16, isOutput=True)
        dbg_pwT = nc.declare_dram_parameter("dbg_pwT", [BPC, C * CG], BF16, isOutput=True)
        dbg_dwb = nc.declare_dram_parameter("dbg_dwb", [128, ROW], BF16, isOutput=True)
        dbg_yp1 = nc.declare_dram_parameter("dbg_yp1", [128, NT, 34, 34], BF16, isOutput=True)
        dbg_biasc = nc.declare_dram_parameter("dbg_biasc", [128, NT, BPC], F32, isOutput=True)

    # one scratch slot per (sample, channel-tile) to keep DMA deps precise
    scr = [[nc.dram_tensor(f"dwscr{b}{t}", [SLOT], BF16) for t in range(NT)]
           for b in range(BPC)]
    # all-to-all staging: 8 blocks x 18 dw rows x 512 cols
    a2ain = nc.dram_tensor("a2ain", [NCORES * 18, NSH], BF16)
    a2aout = nc.dram_tensor("a2aout", [NCORES * 18, NSH], BF16)

    with tile.TileContext(nc) as tc:
        with (
            tc.tile_pool(name="consts", bufs=1) as consts,
            tc.tile_pool(name="stream", bufs=6) as stream,
            tc.tile_pool(name="xin", bufs=2) as xin,
            tc.tile_pool(name="actp", bufs=3) as actp,
            tc.tile_pool(name="wstream", bufs=6) as wstream,
            tc.tile_pool(name="blk", bufs=8) as blkp,
            tc.tile_pool(name="pad3", bufs=5) as pad3,
            tc.tile_pool(name="dwtp", bufs=1) as dwtp,
            tc.tile_pool(name="outp", bufs=2) as outp,
            tc.tile_pool(name="psum", bufs=6, space="PSUM") as psum,
            tc.tile_pool(name="psumb", bufs=2, space="PSUM") as psumb,
        ):
            # ---------------- persistent small constants ----------------
            epsb = consts.tile([128, 1], F32, tag="epsb")
            nc.vector.memset(epsb[:], EPS)
            zbias = consts.tile([128, 1], F32, tag="zbias")
            nc.vector.memset(zbias[:], 0.0)
            onesf = consts.tile([1, 80], F32, tag="onesf")
            nc.vector.memset(onesf[:], 1.0)
            ones = consts.tile([1, 80], BF16, tag="ones")
            nc.vector.tensor_copy(ones[:], onesf[:])
            zbf = consts.tile([128, ROW + 1], BF16, tag="zbf")
            nc.vector.memset(zbf[:], 0.0)

            # zero-fill all scratch slots (issued early, on DVE's DGE)
            for b in range(BPC):
                for t in range(NT):
                    nc.gpsimd.dma_start(
                        out=scr[b][t].rearrange("(p r) -> p r", p=128),
                        in_=zbf[:, :],
                    )

            # ---------------- style im2col / pooled (host-prepared) ----------------
            # xw[p, q, pos, (b,i,j)]: lhsT slices for the dw predictor matmuls
            xw = consts.tile([128, 4, 9, BALL * 9], BF16, tag="xw")
            nc.scalar.dma_start(
                out=xw[:, :, :, :].rearrange("p q pos f -> p q (pos f)"),
                in_=xwh.rearrange("(q p) f -> p q f", p=128),
            )
            pooled = consts.tile([128, 4, BALL], BF16, tag="pooled")
            nc.scalar.dma_start(
                out=pooled[:, :, :], in_=plh.rearrange("(q p) b -> p q b", p=128)
            )
            pooledo = consts.tile([128, 4, BPC], BF16, tag="pooledo")
            nc.scalar.dma_start(
                out=pooledo[:, :, :], in_=plo.rearrange("(q p) b -> p q b", p=128)
            )

            # ------- predictor dw: this core's 512-col slice for ALL 16 samples -------
            # M layout: (sample 16, pos 9) = 144 rows, split into two 72-row psums.
            kswb = consts.tile([1, NSH], BF16, tag="kswb")
            nc.sync.dma_start(out=kswb[:, :], in_=kpsw[9 * S : 9 * S + 1, :])
            psA = psum.tile([72, 512], F32, tag="mm", name="psA")
            psB = psum.tile([72, 512], F32, tag="mm", name="psB")
            for pos in range(9):
                kst = stream.tile([128, 4, 512], BF16, tag="kst")
                nc.sync.dma_start(
                    out=kst[:, :, :],
                    in_=kpsw[pos * S : (pos + 1) * S, :].rearrange(
                        "(q p) c -> p q c", p=128
                    ),
                )
                for q in range(4):
                    first = pos == 0 and q == 0
                    nc.tensor.matmul(
                        psA[:], xw[:, q, pos, :72], kst[:, q, :],
                        start=first, stop=False,
                    )
                    nc.tensor.matmul(
                        psB[:], xw[:, q, pos, 72:144], kst[:, q, :],
                        start=first, stop=False,
                    )
            nc.tensor.matmul(psA[:], ones[:1, :72], kswb[:1, :], start=False, stop=True)
            nc.tensor.matmul(psB[:], ones[:1, :72], kswb[:1, :], start=False, stop=True)

            # stage into all-to-all block layout (18 dw rows per dest block)
            st0 = dwtp.tile([72, 512], BF16, tag="st0")
            st1 = dwtp.tile([72, 512], BF16, tag="st1")
            nc.vector.tensor_copy(st0[:, :], psA[:])
            nc.vector.tensor_copy(st1[:, :], psB[:])
            nc.scalar.dma_start(out=a2ain[:72, :], in_=st0[:, :])
            nc.scalar.dma_start(out=a2ain[72:, :], in_=st1[:, :])

            # exchange: after this, block s of a2aout holds cols [512s, 512s+512)
            # of dw for THIS core's two samples
            nc.gpsimd.collective_compute(
                kind="AllToAll",
                op=AluOpType.bypass,
                replica_groups=[list(range(NCORES))],
                ins=[a2ain[:, :]],
                outs=[a2aout[:, :]],
            )

            # ------- pw predictor: local (own 2 samples, full 4096 cols) -------
            # runs on the PE while the collective is in flight
            kppwb = consts.tile([1, C * CG], BF16, tag="kppwb")
            nc.sync.dma_start(out=kppwb[:, :], in_=kppw[S : S + 1, :])
            pwT = dwtp.tile([BPC, C * CG], BF16, tag="pwT")
            for n in range(8):
                ncol = slice(512 * n, 512 * (n + 1))
                kpt = stream.tile([128, 4, 512], BF16, tag="kst", name=f"kpt{n}")
                nc.sync.dma_start(
                    out=kpt[:, :, :],
                    in_=kppw[:S, ncol].rearrange("(q p) c -> p q c", p=128),
                )
                ps2 = psum.tile([BPC, 512], F32, tag="mm", name=f"pw{n}")
                for q in range(4):
                    nc.tensor.matmul(
                        ps2[:], pooledo[:, q, :], kpt[:, q, :], start=(q == 0), stop=False
                    )
                nc.tensor.matmul(
                    ps2[:], ones[:1, :BPC], kppwb[:1, ncol], start=False, stop=True
                )
                nc.vector.tensor_copy(pwT[:, ncol], ps2[:])

            # bias head (local): biasc[c, t, b] from pooled-own @ kp_bw.T + kp_bb
            kpbwb = consts.tile([1, C], BF16, tag="kpbwb")
            nc.sync.dma_start(out=kpbwb[:, :], in_=kpbw[S : S + 1, :])
            biasc = consts.tile([128, NT, BPC], F32, tag="biasc")
            for m in range(NT):
                kbt = stream.tile([128, 4, 128], BF16, tag="kbt")
                nc.sync.dma_start(
                    out=kbt[:, :, :],
                    in_=kpbw[:S, 128 * m : 128 * (m + 1)].rearrange(
                        "(q p) c -> p q c", p=128
                    ),
                )
                ps = psumb.tile([128, BPC], F32, tag="mmb")
                for q in range(4):
                    nc.tensor.matmul(
                        ps[:], kbt[:, q, :], pooledo[:, q, :], start=(q == 0), stop=False
                    )
                nc.tensor.matmul(
                    ps[:],
                    kpbwb[:1, 128 * m : 128 * (m + 1)],
                    ones[:1, :BPC],
                    start=False,
                    stop=True,
                )
                nc.vector.tensor_copy(biasc[:, m, :], ps[:])

            # ---------------- x load + instance norm -> padded bf16 ----------------
            xps = []
            for b in range(BPC):
                xbig = xin.tile([128, NT, HW], F32, tag="xsb")
                nc.sync.dma_start(
                    out=xbig[:, :, :],
                    in_=x2[b].rearrange("(t p) h w -> p t (h w)", p=128),
                )
                xp = pad3.tile([128, NT, 34, 34], BF16, tag="padbuf")
                xps.append(xp)
                for t in range(NT):
                    st = actp.tile([128, 2, 6], F32, tag="bnst")
                    xsb2 = xbig[:, t, :].rearrange("p (s f) -> p s f", f=512)
                    for sg in range(2):
                        nc.vector.bn_stats(out=st[:, sg, :], in_=xsb2[:, sg, :])
                    mv = actp.tile([128, 2], F32, tag="bnmv")
                    nc.vector.bn_aggr(out=mv[:], in_=st[:])
                    rstd = actp.tile([128, 1], F32, tag="rstd")
                    nc.scalar.activation(
                        out=rstd[:], in_=mv[:, 1:2], func=AF.Sqrt, bias=epsb[:], scale=1.0
                    )
                    nc.vector.reciprocal(out=rstd[:], in_=rstd[:])
                    nc.vector.tensor_scalar(
                        out=xp[:, t, 1:33, 1:33],
                        in0=xbig[:, t, :].rearrange("p (h w) -> p h w", h=H),
                        scalar1=mv[:, 0:1],
                        scalar2=rstd[:],
                        op0=AluOpType.subtract,
                        op1=AluOpType.mult,
                    )
                    nc.vector.tensor_copy(xp[:, t, 1:33, 0:1], xp[:, t, 1:33, 2:3])
                    nc.vector.tensor_copy(xp[:, t, 1:33, 33:34], xp[:, t, 1:33, 31:32])
                    nc.vector.tensor_copy(xp[:, t, 0, :], xp[:, t, 2, :])
                    nc.vector.tensor_copy(xp[:, t, 33, :], xp[:, t, 31, :])

            # assemble dw rows per tile, scatter into the diagonal slots
            dwTP = dwtp.tile([2 * 9, C * CG], BF16, tag="dwtp")
            dwbs = {}
            for t in range(NT):
                tcol = slice(1024 * t, 1024 * (t + 1))
                nc.sync.dma_start(
                    out=dwTP[:, tcol].rearrange("r (s c) -> r s c", s=2),
                    in_=a2aout.rearrange("(s r) c -> r s c", s=NCORES)[
                        :, 2 * t : 2 * t + 2, :
                    ],
                )
                for b in range(BPC):
                    for pos in range(9):
                        eng = nc.sync if pos % 2 == 0 else nc.scalar
                        eng.dma_start(
                            out=_slot_diag_pos(scr[b][t], pos),
                            in_=dwTP[9 * b + pos : 9 * b + pos + 1, tcol].rearrange(
                                "p (g i o) -> p g i o", g=16, i=CG
                            ),
                        )
                    nc.scalar.dma_start(
                        out=_slot_diag_pos(scr[b][t], 9),
                        in_=pwT[b : b + 1, tcol].rearrange(
                            "p (g i o) -> p g i o", g=16, i=CG
                        ),
                    )
                    dwb = blkp.tile([128, ROW], BF16, tag="dwb", name=f"dwb{b}{t}")
                    nc.sync.dma_start(
                        out=dwb[:, :],
                        in_=scr[b][t][: 128 * ROW].rearrange("(p r) -> p r", p=128),
                    )
                    dwbs[(b, t)] = dwb

            if DEBUG:
                nc.sync.dma_start(out=dbg_dwTP[:, :], in_=dwTP[:, :])
                nc.sync.dma_start(out=dbg_pwT[:, :], in_=pwT[:, :])
                nc.sync.dma_start(out=dbg_dwb[:, :], in_=dwbs[(0, 0)][:, :])
                nc.sync.dma_start(out=dbg_biasc[:, :, :], in_=biasc[:, :, :])

            # ---------------- phase A: adaconv (dynamic grouped conv + pointwise) ----
            yp1s = []
            for b in range(BPC):
                yp1 = pad3.tile([128, NT, 34, 34], BF16, tag="padbuf")
                yp1s.append(yp1)
                for t in range(NT):
                    nc.vector.tensor_copy(yp1[:, t, 0, :], zbf[:, :34])
                    nc.vector.tensor_copy(yp1[:, t, 33, :], zbf[:, :34])
                    nc.vector.tensor_copy(
                        yp1[:, t, 1:33, 0:1],
                        zbf[:, :32].rearrange("p (a c) -> p a c", c=1),
                    )
                    nc.vector.tensor_copy(
                        yp1[:, t, 1:33, 33:34],
                        zbf[:, :32].rearrange("p (a c) -> p a c", c=1),
                    )
            for t in range(NT):
                for b in range(BPC):
                    dwb = dwbs[(b, t)]
                    ysb = actp.tile([128, HW], BF16, tag="ysb")
                    for hh in range(2):
                        ps = psum.tile([128, 512], F32, tag="mm")
                        for kdi in range(3):
                            for kdj in range(3):
                                pos = kdi * 3 + kdj
                                nc.tensor.matmul(
                                    ps[:],
                                    dwb[:, 129 * pos : 129 * pos + 128],
                                    xps[b][:, t, kdi + 16 * hh : kdi + 16 * hh + 16, kdj : kdj + 32],
                                    start=(pos == 0),
                                    stop=(pos == 8),
                                )
                        nc.vector.tensor_copy(ysb[:, 512 * hh : 512 * (hh + 1)], ps[:])
                    for hh in range(2):
                        ps2 = psum.tile([128, 512], F32, tag="mm")
                        nc.tensor.matmul(
                            ps2[:],
                            dwb[:, 129 * 9 : 129 * 9 + 128],
                            ysb[:, 512 * hh : 512 * (hh + 1)],
                            start=True,
                            stop=True,
                        )
                        nc.scalar.activation(
                            out=yp1s[b][:, t, 1 + 16 * hh : 17 + 16 * hh, 1:33],
                            in_=ps2[:].rearrange("p (h w) -> p h w", h=16),
                            func=AF.Identity,
                            bias=biasc[:, t, b : b + 1],
                            scale=1.0,
                        )

            if DEBUG:
                nc.sync.dma_start(out=dbg_yp1[:, :, :, :], in_=yp1s[0][:, :, :, :])

            b1sb = consts.tile([128, NT], F32, tag="b1sb")
            nc.sync.dma_start(out=b1sb[:, :], in_=b1d.rearrange("(m c) -> c m", c=128))
            b2sb = consts.tile([128, NM2], F32, tag="b2sb")
            nc.sync.dma_start(out=b2sb[:, :], in_=b2d.rearrange("(m c) -> c m", c=128))

            # ---------------- phase B: conv1 (512 -> 512) + relu ----------------
            yp2s = []
            for b in range(BPC):
                yp2 = pad3.tile([128, NT, 34, 34], BF16, tag="padbuf")
                yp2s.append(yp2)
                for m in range(NT):
                    nc.vector.tensor_copy(yp2[:, m, 0, :], zbf[:, :34])
                    nc.vector.tensor_copy(yp2[:, m, 33, :], zbf[:, :34])
                    nc.vector.tensor_copy(
                        yp2[:, m, 1:33, 0:1],
                        zbf[:, :32].rearrange("p (a c) -> p a c", c=1),
                    )
                    nc.vector.tensor_copy(
                        yp2[:, m, 1:33, 33:34],
                        zbf[:, :32].rearrange("p (a c) -> p a c", c=1),
                    )
            for b in range(BPC):
                for m in range(NT):
                    pss = [psum.tile([128, 512], F32, tag="mm", name=f"pss{b}_{m}_{i}") for i in range(2)]
                    for k in range(NT):
                        w1k = wstream.tile([128, 9 * 128], BF16, tag="ws")
                        nc.sync.dma_start(out=w1k[:, :], in_=w1p[k, m])
                        for hh in range(2):
                            ps = pss[hh]
                            for kdi in range(3):
                                for kdj in range(3):
                                    pos = kdi * 3 + kdj
                                    nc.tensor.matmul(
                                        ps[:],
                                        w1k[:, 128 * pos : 128 * (pos + 1)],
                                        yp1s[b][:, k, kdi + 16 * hh : kdi + 16 * hh + 16, kdj : kdj + 32],
                                        start=(k == 0 and pos == 0),
                                        stop=(k == NT - 1 and pos == 8),
                                    )
                    for hh in range(2):
                        nc.scalar.activation(
                            out=yp2s[b][:, m, 1 + 16 * hh : 17 + 16 * hh, 1:33],
                            in_=pss[hh][:].rearrange("p (h w) -> p h w", h=16),
                            func=AF.Relu,
                            bias=b1sb[:, m : m + 1],
                            scale=1.0,
                        )

            # ------- phase C: conv2 (512 -> 256) + relu + 2x nearest upsample -------
            for b in range(BPC):
                for m2 in range(NM2):
                    pss = [psum.tile([128, 512], F32, tag="mm", name=f"psc{b}_{m2}_{i}") for i in range(2)]
                    for k in range(NT):
                        w2k = wstream.tile([128, 9 * 128], BF16, tag="ws")
                        nc.sync.dma_start(out=w2k[:, :], in_=w2p[k, m2])
                        for hh in range(2):
                            ps = pss[hh]
                            for kdi in range(3):
                                for kdj in range(3):
                                    pos = kdi * 3 + kdj
                                    nc.tensor.matmul(
                                        ps[:],
                                        w2k[:, 128 * pos : 128 * (pos + 1)],
                                        yp2s[b][:, k, kdi + 16 * hh : kdi + 16 * hh + 16, kdj : kdj + 32],
                                        start=(k == 0 and pos == 0),
                                        stop=(k == NT - 1 and pos == 8),
                                    )
                    for hh in range(2):
                        ps = pss[hh]
                        # ous free dim = (h 16, two_h 2, w 32, two_w 2) = 2048
                        ous = outp.tile([128, 16, 2, 32, 2], F32, tag="ous")
                        for a in range(2):
                            for a2 in range(2):
                                nc.scalar.activation(
                                    out=ous[:, :, a, :, a2],
                                    in_=ps[:].rearrange("p (h w) -> p h w", h=16),
                                    func=AF.Relu,
                                    bias=b2sb[:, m2 : m2 + 1],
                                    scale=1.0,
                                )
                        nc.sync.dma_start(
                            out=yout[b, 128 * m2 : 128 * (m2 + 1)]
                            .rearrange("c h w -> c (h w)")[
                                :, 2048 * hh : 2048 * (hh + 1)
                            ]
                            .rearrange("c (h r) -> c h r", h=16),
                            in_=ous[:, :, :, :, :].rearrange("p h th w tw -> p h (th w tw)"),
                        )

    nc.compile()
    return nc


def _repack(inputs):
    bf = ml_dtypes.bfloat16
    kp_sw = np.ascontiguousarray(inputs["kp_sw"], dtype=np.float32)
    kp_sb = np.ascontiguousarray(inputs["kp_sb"], dtype=np.float32)
    kp_pw = np.ascontiguousarray(inputs["kp_pw"], dtype=np.float32)
    kp_pb = np.ascontiguousarray(inputs["kp_pb"], dtype=np.float32)
    kp_bw = np.ascontiguousarray(inputs["kp_bw"], dtype=np.float32)
    kp_bb = np.ascontiguousarray(inputs["kp_bb"], dtype=np.float32)
    dec_w1 = np.ascontiguousarray(inputs["dec_w1"], dtype=np.float32)
    dec_b1 = np.ascontiguousarray(inputs["dec_b1"], dtype=np.float32)
    dec_w2 = np.ascontiguousarray(inputs["dec_w2"], dtype=np.float32)
    dec_b2 = np.ascontiguousarray(inputs["dec_b2"], dtype=np.float32)

    # column permutation: new col (t, g, i, o) <- original row (g*8+o)*8 + i
    O = np.arange(C * CG).reshape(NT, 16, CG, CG)  # (t, g, o, i), flat-major
    P = O.transpose(0, 1, 3, 2).reshape(-1)        # (t, g, i, o)

    kpsw = np.empty((9 * S + 1, C * CG), dtype=bf)
    kpsw[: 9 * S] = (
        kp_sw[P].reshape(C * CG, S, 3, 3).transpose(2, 3, 1, 0).reshape(9 * S, C * CG)
    ).astype(bf)  # rows in k-order (di, dj, s)
    kpsw[9 * S] = kp_sb[P].astype(bf)
    kpsw_sl = [np.ascontiguousarray(kpsw[:, NSH * c : NSH * (c + 1)])
               for c in range(NCORES)]

    kppw = np.empty((S + 1, C * CG), dtype=bf)
    kppw[:S] = kp_pw[P].T.astype(bf)
    kppw[S] = kp_pb[P].astype(bf)

    kpbw = np.empty((S + 1, C), dtype=bf)
    kpbw[:S] = kp_bw.T.astype(bf)
    kpbw[S] = kp_bb.astype(bf)

    # decoder weights: [k, m, p, (pos, c)] fully contiguous per (k, m) tile
    w1t = dec_w1.transpose(2, 3, 1, 0).reshape(9, NT, 128, NT, 128)
    w1p = np.ascontiguousarray(w1t.transpose(1, 3, 2, 0, 4).reshape(NT, NT, 128, 9 * 128)).astype(bf)
    w2t = dec_w2.transpose(2, 3, 1, 0).reshape(9, NT, 128, NM2, 128)
    w2p = np.ascontiguousarray(w2t.transpose(1, 3, 2, 0, 4).reshape(NT, NM2, 128, 9 * 128)).astype(bf)

    shared = {
        "kppw": kppw,
        "kpbw": kpbw,
        "w1p": w1p,
        "w2p": w2p,
        "b1d": dec_b1,
        "b2d": dec_b2,
    }
    return shared, kpsw_sl


def _style_prep(w):
    """Host-side style-map im2col (reflect-padded) + pooled means, bf16."""
    bf = ml_dtypes.bfloat16
    wf = np.ascontiguousarray(w, dtype=np.float32)
    wpad = np.pad(wf, ((0, 0), (0, 0), (1, 1), (1, 1)), mode="reflect")
    xwh = np.empty((S, 9, BALL * 9), np.float32)
    for di in range(3):
        for dj in range(3):
            win = wpad[:, :, di : di + 3, dj : dj + 3]
            xwh[:, di * 3 + dj, :] = win.transpose(1, 0, 2, 3).reshape(S, BALL * 9)
    pl = np.ascontiguousarray(wf.mean(axis=(2, 3)).T)
    return (
        np.ascontiguousarray(xwh.reshape(S, 9 * BALL * 9)).astype(bf),
        pl.astype(bf),
    )


def kernel(**inputs):
    if "nc" not in _CACHE:
        _CACHE["nc"] = _build()
    nc = _CACHE["nc"]

    shared, kpsw_sl = _repack(inputs)
    x = np.ascontiguousarray(inputs["x"], dtype=np.float32)
    w = np.ascontiguousarray(inputs["w"], dtype=np.float32)
    xwh, pl = _style_prep(w)

    in_maps = []
    for c in range(NCORES):
        sl = slice(BPC * c, BPC * (c + 1))
        in_maps.append({
            "x2": x[sl], "xwh": xwh, "plh": pl,
            "plo": np.ascontiguousarray(pl[:, sl]),
            "kpsw": kpsw_sl[c], **shared,
        })

    res = run_bass_kernel_spmd(nc, in_maps, list(range(NCORES))).results
    return np.concatenate([r["yout"] for r in res], axis=0)
